# revision 1
# baseline (speedup 1.0000x reference)
"""Distributed MultiHeadAttention kernel for 8 Trainium2 NeuronCores.

Problem: B=2, L=2048, D=1024, H=16 heads (DH=64), causal attn_mask +
key_padding_mask, torch-Linear-convention projections.

Sharding: core = (batch b = core//4, group rank j = core%4). Each core
projects q/k/v for its batch restricted to its 4 heads (256 channels),
runs streaming softmax attention in a [key, query]-transposed layout
(no max subtraction -- scores are O(1); masked scores get -1e5 added so
exp underflows to exactly 0). Projections (phase P) are FUSED with
attention (phase A): each 512-position projection chunk unblocks the
matching attention segment of head-pair 0, so the exp stream (the ACT
engine, the densest per-engine cost at ~80us) starts ~25us into the
kernel instead of after all projections.

The causal mask is applied structurally (per 128-key block only
queries >= block start are computed) plus a constant [128,128]
lower-triangle additive tile on diagonal blocks; key-padding rides the
exp ACTIVATE's per-partition bias for free. Both heads of a pair issue
score matmuls to different row groups + PSUM banks (concurrent in the
PE array), and one merged strided ACTIVATE exps both heads' scores.

Softmax division happens SENDER-side: 1/S via fast-approx reciprocal,
broadcast across the 64 channel partitions with a one-hot selector
matmul (padded to K=32: a K=8 bf16 matmul contracts the full 32-row
group on HW -- stale weights x SBUF garbage; sim won't show this),
then one AllGather per head-pair ships normalized attention (no S
collective at all). Receivers slice their own 512 queries via
partition-id-indexed DMA and run the output projection; the p=0
partial o_proj overlaps the p=1 AllGather, with PE heater matmuls
keeping the HAM clock gate warm across both collective waits.

Matmuls run in bf16 (fp32 PE matmul is 4x slower); accumulation fp32.
Inputs are transposed to [D, L] on the host (DMA-transpose serializes
on the xbar queue; host transpose is free on the device timeline).
"""
import os
import sys

sys.path.insert(0, '/opt/trn_rl_repo')

import numpy as np
import ml_dtypes

import concourse.bass as bass
import concourse.bacc as bacc
import concourse.mybir as mybir
import concourse.tile as tile
from concourse.bass_utils import run_bass_kernel_spmd

BF16 = mybir.dt.bfloat16
F32 = mybir.dt.float32
NPBF16 = ml_dtypes.bfloat16

B, L, D, H = 2, 2048, 1024, 16
DH = D // H                      # 64
N_CORES = 8
GROUPS = [[0, 1, 2, 3], [4, 5, 6, 7]]
HPC = H // 4                     # heads per core = 4
CPC = HPC * DH                   # channels per core = 256
LPC = L // 4                     # output rows per core = 512
SEG = 512                        # query segment (= per-rank slice)
NSEG = L // SEG                  # 4
KB = 128                         # key-block size
NKB = L // KB                    # 16
MASK_VAL = -1e5                  # exp(MASK_VAL/8 + s) == 0 in fp32
NDB = D // 128                   # 8 contraction blocks

ExpFn = mybir.ActivationFunctionType.Exp

_PROG_CACHE = {}
last_results = None


def _build_program():
    """Build the SPMD Bass program (identical on all 8 cores)."""
    nc = bacc.Bacc("TRN2", target_bir_lowering=False, debug=False,
                   num_devices=N_CORES)

    xqT = nc.declare_dram_parameter("xqT", [D, L], BF16, isOutput=False)
    xkT = nc.declare_dram_parameter("xkT", [D, L], BF16, isOutput=False)
    xvT = nc.declare_dram_parameter("xvT", [D, L], BF16, isOutput=False)
    wqT = nc.declare_dram_parameter("wqT", [D, CPC], BF16, isOutput=False)
    wkT = nc.declare_dram_parameter("wkT", [D, CPC], BF16, isOutput=False)
    wvT = nc.declare_dram_parameter("wvT", [D, CPC], BF16, isOutput=False)
    woT = nc.declare_dram_parameter("woT", [D, D], BF16, isOutput=False)
    bq_in = nc.declare_dram_parameter("bq", [128, 2], F32, isOutput=False)
    bk_in = nc.declare_dram_parameter("bk", [128, 2], F32, isOutput=False)
    bv_in = nc.declare_dram_parameter("bv", [1, CPC], BF16, isOutput=False)
    bo_in = nc.declare_dram_parameter("bo", [1, D], BF16, isOutput=False)
    pad_in = nc.declare_dram_parameter("pad", [128, NKB], F32, isOutput=False)
    tri_in = nc.declare_dram_parameter("tri", [128, 128], F32, isOutput=False)
    sels_in = nc.declare_dram_parameter("sels", [32, 8, 64], BF16,
                                        isOutput=False)
    out = nc.declare_dram_parameter("out", [LPC, D], F32, isOutput=True)

    # per head-pair AllGather bounce buffers: rows = NORMALIZED attn
    # (hp0 ch 0-63, hp1 ch 64-127); softmax division happens sender-side
    # (hidden under the CC stream wait), so no S data ships at all.
    ag_in = [nc.dram_tensor(f"ag_in{p}", [128, L], BF16) for p in range(2)]
    ag_out = [nc.dram_tensor(f"ag_out{p}", [4, 128, L], BF16)
              for p in range(2)]

    with tile.TileContext(nc, num_cores=N_CORES) as tc:
        with tc.tile_pool(name="persist", bufs=1) as pers:
            wq_sb = pers.tile([128, NDB, CPC], BF16, tag="wq")
            wk_sb = pers.tile([128, NDB, CPC], BF16, tag="wk")
            wv_sb = pers.tile([128, NDB, CPC], BF16, tag="wv")
            wo_sb = pers.tile([128, NDB, D], BF16, tag="wo")
            bq_sb = pers.tile([128, 2], F32, tag="bq")
            bk_sb = pers.tile([128, 2], F32, tag="bk")
            bv_sb = pers.tile([1, CPC], BF16, tag="bv")
            bo_sb = pers.tile([1, D], BF16, tag="bo")
            pad_sb = pers.tile([128, NKB], F32, tag="pad")
            tri_sb = pers.tile([128, 128], F32, tag="tri")
            ones_sb = pers.tile([1, 128], BF16, tag="ones")
            sels_sb = pers.tile([32, 8, 64], BF16, tag="sels")
            qT_sb = pers.tile([128, 2, L], BF16, tag="qT")
            kT_sb = pers.tile([128, 2, L], BF16, tag="kT")
            v_sb = pers.tile([128, NKB, HPC, DH + 1], BF16, tag="v")
            dumm_sb = pers.tile([1, 1], BF16, tag="dumm")

            # spread big loads across three DMA queues; small constants go
            # on the (idle-early) vector queue so they never delay x-chunks
            nc.sync.dma_start(
                out=wq_sb[:], in_=wqT.ap().rearrange("(db p) c -> p db c", p=128))
            nc.scalar.dma_start(
                out=wk_sb[:], in_=wkT.ap().rearrange("(db p) c -> p db c", p=128))
            nc.gpsimd.dma_start(
                out=wv_sb[:], in_=wvT.ap().rearrange("(db p) c -> p db c", p=128))
            nc.gpsimd.dma_start(out=bq_sb[:], in_=bq_in[:])
            nc.gpsimd.dma_start(out=bk_sb[:], in_=bk_in[:])
            nc.gpsimd.dma_start(out=bv_sb[:], in_=bv_in[:])
            nc.gpsimd.dma_start(out=bo_sb[:], in_=bo_in[:])
            nc.gpsimd.dma_start(out=pad_sb[:], in_=pad_in[:])
            nc.gpsimd.dma_start(out=tri_sb[:], in_=tri_in[:])
            nc.vector.memset(ones_sb[:], 1.0)
            nc.gpsimd.dma_start(out=sels_sb[:], in_=sels_in[:])
            nc.vector.memset(v_sb[:, :, :, DH:DH + 1], 1.0)
            # receiver-side query-slice offsets (per engine registers)
            l0r_e = {}
            for _eng in (nc.sync, nc.scalar):
                l0r_e[_eng.engine] = (_eng.partition_id() % 4) * 512
            # preload the exp table set during the input DMA wait
            nc.vector.memset(dumm_sb[:], 0.0)
            nc.scalar.activation(out=dumm_sb[:], in_=dumm_sb[:], func=ExpFn)
            # PE heater: warm the HAM clock gate while input DMAs stream;
            # sized to bridge the first x-chunk landing (~10us), no more.
            heat_sb = pers.tile([128, 1024], BF16, tag="heat")
            nc.vector.memset(heat_sb[:], 0.001)
            with tc.tile_pool(name="psH", bufs=1, space="PSUM") as psH:
                hps = psH.tile([128, 512], F32, tag="hps")
                for it in range(32):
                    nc.tensor.matmul(hps[:], lhsT=heat_sb[:, 0:128],
                                     rhs=heat_sb[:, 512:1024],
                                     start=(it == 0), stop=(it == 31))

            # ------- Fused phases P+A: projections interleaved with ------
            # ------- attention so the exp (ACT) stream starts early ------
            ctxPA = nc.named_scope("phasePA"); ctxPA.__enter__()
            with tc.tile_pool(name="xt", bufs=2) as xtp, \
                 tc.tile_pool(name="psP", bufs=2, space="PSUM") as psP, \
                 tc.tile_pool(name="ex", bufs=4) as exp_pool, \
                 tc.tile_pool(name="araw", bufs=2) as arawp, \
                 tc.tile_pool(name="sm", bufs=4) as smalls, \
                 tc.tile_pool(name="psX", bufs=2, space="PSUM") as psX, \
                 tc.tile_pool(name="psA", bufs=2, space="PSUM") as psA:

                def do_P(lc):
                    l0 = lc * 512
                    xtq = xtp.tile([128, NDB, 512], BF16, tag="xtq")
                    xtk = xtp.tile([128, NDB, 512], BF16, tag="xtk")
                    xtv = xtp.tile([128, NDB, 512], BF16, tag="xtv")
                    nc.sync.dma_start(
                        out=xtq[:],
                        in_=xqT.ap().rearrange("(db p) l -> p db l", p=128)
                        [:, :, l0:l0 + 512])
                    nc.scalar.dma_start(
                        out=xtk[:],
                        in_=xkT.ap().rearrange("(db p) l -> p db l", p=128)
                        [:, :, l0:l0 + 512])
                    nc.gpsimd.dma_start(
                        out=xtv[:],
                        in_=xvT.ap().rearrange("(db p) l -> p db l", p=128)
                        [:, :, l0:l0 + 512])
                    for (w_sb, b_sb, t_sb, x_sb) in ((wq_sb, bq_sb, qT_sb, xtq),
                                                     (wk_sb, bk_sb, kT_sb, xtk)):
                        for cb in range(2):
                            ps = psP.tile([128, 512], F32, tag="psP",
                                          name=f"ps_{lc}_{cb}")
                            for db in range(NDB):
                                nc.tensor.matmul(
                                    ps[:],
                                    lhsT=w_sb[:, db, cb * 128:(cb + 1) * 128],
                                    rhs=x_sb[:, db, :],
                                    start=(db == 0), stop=(db == NDB - 1))
                            nc.vector.tensor_scalar_add(
                                t_sb[:, cb, l0:l0 + 512], ps[:],
                                b_sb[:, cb:cb + 1])
                    for ls in range(4):
                        kbg = lc * 4 + ls
                        psv = psP.tile([128, CPC], F32, tag="psP",
                                       name=f"psv_{lc}_{ls}")
                        for db in range(NDB):
                            nc.tensor.matmul(
                                psv[:],
                                lhsT=xtv[:, db, ls * 128:(ls + 1) * 128],
                                rhs=wv_sb[:, db, :],
                                start=(db == 0), stop=False)
                        nc.tensor.matmul(
                            psv[:], lhsT=ones_sb[:, 0:128], rhs=bv_sb[:],
                            start=False, stop=True)
                        nc.vector.tensor_copy(
                            v_sb[:, kbg, :, 0:DH],
                            psv[:].rearrange("p (h d) -> p h d", h=HPC))

                def alloc_p(p):
                    araw = arawp.tile([64, 2, NSEG, SEG], BF16, tag="araw",
                                      name=f"araw_{p}")
                    fn = arawp.tile([64, 2, NSEG, SEG], BF16, tag="fn",
                                    name=f"fn_{p}")
                    s_sb = smalls.tile([8, 512], F32, tag="sall",
                                       name=f"sall_{p}")
                    return araw, fn, s_sb

                def do_A_seg(p, seg, araw, s_sb):
                    nkb = (seg + 1) * 4
                    pa = {hp: psA.tile([65, SEG], F32, tag="pa",
                                       name=f"pa_{p}_{seg}_{hp}")
                          for hp in range(2)}
                    for kb in range(nkb):
                        o = max(0, kb * KB - seg * SEG)
                        ps = psX.tile([128, 2, SEG], F32, tag="psX",
                                      name=f"ps_{p}_{seg}_{kb}")
                        for hp in range(2):
                            hoff = hp * 64
                            # both heads of the pair target different
                            # row groups + PSUM banks -> concurrent MMs
                            nc.tensor.matmul(
                                ps[:, hp, o:SEG],
                                lhsT=kT_sb[hoff:hoff + 64, p,
                                           kb * KB:(kb + 1) * KB],
                                rhs=qT_sb[hoff:hoff + 64, p,
                                          seg * SEG + o:(seg + 1) * SEG],
                                start=True, stop=True)
                        if kb >= seg * 4:  # diagonal block: causal tri
                            for hp in range(2):
                                nc.vector.tensor_add(
                                    ps[:, hp, o:o + 128],
                                    ps[:, hp, o:o + 128], tri_sb[:])
                        ex = exp_pool.tile([128, 2, SEG], BF16, tag="ex",
                                           name=f"ex_{p}_{seg}_{kb}")
                        nc.scalar.activation(
                            out=ex[:, :, o:], in_=ps[:, :, o:], func=ExpFn,
                            scale=0.125, bias=pad_sb[:, kb:kb + 1])
                        for hp in range(2):
                            h = p * 2 + hp
                            nc.tensor.matmul(
                                pa[hp][:, o:], lhsT=v_sb[:, kb, h, :],
                                rhs=ex[:, hp, o:],
                                start=(kb == 0), stop=(kb == nkb - 1))
                    # evacuate: numerator rows 0..63 -> araw (bf16);
                    # S row 64 -> s_sb row hp*4+seg (DMA moves it
                    # across partitions; engines cannot)
                    for hp in range(2):
                        idx = hp * 4 + seg
                        nc.vector.tensor_copy(
                            araw[:, hp, seg, :], pa[hp][0:64, :])
                        stmp = smalls.tile(
                            [65, SEG], F32, tag="stmp",
                            name=f"stmp_{p}_{seg}_{hp}")
                        nc.vector.tensor_copy(
                            stmp[64:65, :], pa[hp][64:65, :])
                        nc.gpsimd.dma_start(
                            out=s_sb[idx:idx + 1, :],
                            in_=stmp[64:65, :])

                def do_norm_ag(p, araw, fn, s_sb):
                    # sender-side softmax division: 1/S (fast approx, S is
                    # bounded positive), broadcast each row to 64 channel
                    # partitions via a one-hot bf16 matmul, multiply, ship.
                    r16f = smalls.tile([8, 512], F32, tag="r16f",
                                       name=f"r16f_{p}")
                    r16b = smalls.tile([32, 512], BF16, tag="r16b",
                                       name=f"r16b_{p}")
                    nc.vector.reciprocal_approx_fast(r16f[:], s_sb[:])
                    nc.vector.memset(r16b[:], 0.0)
                    nc.vector.tensor_copy(r16b[0:8, :], r16f[:])
                    for seg in range(NSEG):
                        for hp in range(2):
                            idx = hp * 4 + seg
                            bc_ps = psA.tile([65, SEG], F32, tag="pa",
                                             name=f"bc_{p}_{seg}_{hp}")
                            nc.tensor.matmul(
                                bc_ps[0:64, :],
                                lhsT=sels_sb[:, idx, :],
                                rhs=r16b[:],
                                start=True, stop=True)
                            nc.vector.tensor_mul(
                                fn[:, hp, seg, :], araw[:, hp, seg, :],
                                bc_ps[0:64, :])
                        nc.sync.dma_start(
                            out=ag_in[p].ap()[:, seg * SEG:(seg + 1) * SEG]
                            .rearrange("(hp c) l -> c hp l", hp=2),
                            in_=fn[:, :, seg, :])
                    nc.gpsimd.collective_compute(
                        "AllGather", mybir.AluOpType.bypass,
                        replica_groups=GROUPS,
                        ins=[ag_in[p][:]], outs=[ag_out[p][:]])

                do_P(0)
                do_P(1)
                araw0, fn0, s0 = alloc_p(0)
                do_A_seg(0, 0, araw0, s0)
                do_P(2)
                do_A_seg(0, 1, araw0, s0)
                do_P(3)
                nc.scalar.dma_start(
                    out=wo_sb[:],
                    in_=woT.ap().rearrange("(db p) c -> p db c", p=128))
                do_A_seg(0, 2, araw0, s0)
                do_A_seg(0, 3, araw0, s0)
                do_norm_ag(0, araw0, fn0, s0)
                araw1, fn1, s1 = alloc_p(1)
                for seg in range(NSEG):
                    do_A_seg(1, seg, araw1, s1)
                do_norm_ag(1, araw1, fn1, s1)

            ctxPA.__exit__(None, None, None)
            # ---------------- Phase O: output projection ----------------
            ctxO = nc.named_scope("phaseO"); ctxO.__enter__()
            # stage 0 (p=0 partials) runs during AG(p1); its results park in
            # SBUF so PSUM stays mostly free and heaters can keep the PE
            # clock warm across both collective waits.
            with tc.tile_pool(name="psH2", bufs=1, space="PSUM") as psH2, \
                 tc.tile_pool(name="fat", bufs=1) as fatp, \
                 tc.tile_pool(name="ob", bufs=3) as obp, \
                 tc.tile_pool(name="psO", bufs=3, space="PSUM") as psO:
                hps2 = psH2.tile([128, 128], F32, tag="hps2")
                for it in range(160):
                    nc.tensor.matmul(hps2[:], lhsT=heat_sb[:, 0:128],
                                     rhs=heat_sb[:, 512:640],
                                     start=(it == 0), stop=(it == 159))
                fatn = []
                for p in range(2):
                    # own l-slice of rank r's heads (4r+2p, 4r+2p+1);
                    # rows already in channel order, already normalized.
                    fat = fatp.tile([128, 4, 512], BF16, tag=f"fat{p}",
                                    name=f"fat_{p}")
                    eng = nc.sync if p == 0 else nc.scalar
                    l0e = l0r_e[eng.engine]
                    for r in range(4):
                        eng.dma_start(
                            out=fat[:, r, :],
                            in_=ag_out[p][r, :, bass.ds(l0e, 512)])
                    fatn.append(fat)
                sb0 = fatp.tile([128, 4, 2, 512], F32, tag="sb0")
                for ls in range(4):
                    for nch in range(2):
                        po = psO.tile([128, 512], F32, tag="po",
                                      name=f"po0_{ls}_{nch}")
                        for r in range(4):
                            nc.tensor.matmul(
                                po[:],
                                lhsT=fatn[0][:, r, ls * 128:(ls + 1) * 128],
                                rhs=wo_sb[:, r * 2,
                                          nch * 512:(nch + 1) * 512],
                                start=(r == 0), stop=(r == 3))
                        nc.vector.tensor_copy(sb0[:, ls, nch, :], po[:])
                # keep the PE warm while AG(p1) drains
                hps3 = psH2.tile([128, 128], F32, tag="hps2", name="hps3")
                for it in range(160):
                    nc.tensor.matmul(hps3[:], lhsT=heat_sb[:, 0:128],
                                     rhs=heat_sb[:, 512:640],
                                     start=(it == 0), stop=(it == 159))
                for ls in range(4):
                    for nch in range(2):
                        po = psO.tile([128, 512], F32, tag="po",
                                      name=f"po1_{ls}_{nch}")
                        for r in range(4):
                            nc.tensor.matmul(
                                po[:],
                                lhsT=fatn[1][:, r, ls * 128:(ls + 1) * 128],
                                rhs=wo_sb[:, r * 2 + 1,
                                          nch * 512:(nch + 1) * 512],
                                start=(r == 0), stop=False)
                        nc.tensor.matmul(
                            po[:], lhsT=ones_sb[:, 0:128],
                            rhs=bo_sb[:, nch * 512:(nch + 1) * 512],
                            start=False, stop=True)
                        ob = obp.tile([128, 512], F32, tag="ob",
                                      name=f"ob_{ls}_{nch}")
                        nc.vector.tensor_add(ob[:], po[:],
                                             sb0[:, ls, nch, :])
                        nc.sync.dma_start(
                            out=out[ls * 128:(ls + 1) * 128,
                                    nch * 512:(nch + 1) * 512],
                            in_=ob[:])

    ctxO.__exit__(None, None, None)
    nc.compile()
    return nc


def _check_masks(attn_mask, key_padding_mask):
    """The fast path handles exactly-causal attn_mask plus any key padding
    that leaves key 0 unpadded (no all-masked softmax rows)."""
    causal = np.triu(np.ones((L, L), np.bool_), k=1)
    if not np.array_equal(attn_mask, causal):
        return None, True
    if key_padding_mask[:, 0].any():
        return None, True
    pad = [np.ascontiguousarray(
        np.where(key_padding_mask[b].reshape(NKB, KB).T,
                 np.float32(MASK_VAL), np.float32(0.0)))
           for b in range(B)]                              # [128, NKB]
    return pad, False


def _host_fallback(query, key, value, attn_mask, key_padding_mask,
                   Wq, bq, Wk, bk, Wv, bv, Wo, bo):
    """Exact fp32 numpy replica of the reference (degenerate masks only)."""
    q = (query @ Wq.T + bq).reshape(B, L, H, DH).transpose(0, 2, 1, 3)
    k = (key @ Wk.T + bk).reshape(B, L, H, DH).transpose(0, 2, 1, 3)
    v = (value @ Wv.T + bv).reshape(B, L, H, DH).transpose(0, 2, 1, 3)
    scores = np.einsum('bhqd,bhkd->bhqk', q, k) / np.sqrt(np.float32(DH))
    scores = np.where(key_padding_mask[:, None, None, :], -1e30, scores)
    scores = np.where(attn_mask[None, None, :, :], -1e30, scores)
    scores = scores - scores.max(axis=-1, keepdims=True)
    w = np.exp(scores)
    w = w / w.sum(axis=-1, keepdims=True)
    attn = np.einsum('bhqk,bhkd->bhqd', w, v)
    attn = attn.transpose(0, 2, 1, 3).reshape(B, L, D)
    return (attn @ Wo.T + bo).astype(np.float32)


def kernel(query, key, value, attn_mask, key_padding_mask,
           Wq, bq, Wk, bk, Wv, bv, Wo, bo):
    global last_results
    query = np.asarray(query, dtype=np.float32)
    key = np.asarray(key, dtype=np.float32)
    value = np.asarray(value, dtype=np.float32)
    attn_mask = np.asarray(attn_mask, dtype=bool)
    key_padding_mask = np.asarray(key_padding_mask, dtype=bool)
    Wq, bq = np.asarray(Wq, np.float32), np.asarray(bq, np.float32)
    Wk, bk = np.asarray(Wk, np.float32), np.asarray(bk, np.float32)
    Wv, bv = np.asarray(Wv, np.float32), np.asarray(bv, np.float32)
    Wo, bo = np.asarray(Wo, np.float32), np.asarray(bo, np.float32)

    pad_bufs, degenerate = _check_masks(attn_mask, key_padding_mask)
    if degenerate:
        return _host_fallback(query, key, value, attn_mask, key_padding_mask,
                              Wq, bq, Wk, bk, Wv, bv, Wo, bo)

    if "prog" not in _PROG_CACHE:
        _PROG_CACHE["prog"] = _build_program()
    nc = _PROG_CACHE["prog"]

    tri = np.where(np.arange(128)[None, :] < np.arange(128)[:, None],
                   np.float32(MASK_VAL), np.float32(0.0))   # [k', q']
    sels = np.zeros((32, 8, 64), np.float32)
    for _i in range(8):
        sels[_i, _i, :] = 1.0
    sels = sels.astype(NPBF16)
    woT_np = np.ascontiguousarray(Wo.T).astype(NPBF16)
    bo_np = bo.reshape(1, D).astype(NPBF16)
    xT_bf = [np.ascontiguousarray(a.transpose(0, 2, 1)).astype(NPBF16)
             for a in (query, key, value)]             # [B, D, L] bf16

    in_maps = []
    for core in range(N_CORES):
        b, j = divmod(core, 4)
        csl = slice(j * CPC, (j + 1) * CPC)
        in_maps.append({
            "xqT": xT_bf[0][b],
            "xkT": xT_bf[1][b],
            "xvT": xT_bf[2][b],
            "wqT": np.ascontiguousarray(Wq[csl, :].T).astype(NPBF16),
            "wkT": np.ascontiguousarray(Wk[csl, :].T).astype(NPBF16),
            "wvT": np.ascontiguousarray(Wv[csl, :].T).astype(NPBF16),
            "woT": woT_np,
            "bq": np.ascontiguousarray(bq[csl].reshape(2, 128).T),
            "bk": np.ascontiguousarray(bk[csl].reshape(2, 128).T),
            "bv": bv[csl].reshape(1, CPC).astype(NPBF16),
            "bo": bo_np,
            "pad": pad_bufs[b],
            "tri": tri,
            "sels": sels,
        })

    trace = os.environ.get("KERNEL_TRACE", "0") == "1"
    res = run_bass_kernel_spmd(nc, in_maps, list(range(N_CORES)), trace=trace)
    last_results = res

    out = np.empty((B, L, D), dtype=np.float32)
    for core in range(N_CORES):
        b, j = divmod(core, 4)
        out[b, j * LPC:(j + 1) * LPC, :] = res.results[core]["out"]
    return out

